# revision 1
# baseline (speedup 1.0000x reference)
"""Differential Multi-Query Attention — TRN2 Bass kernel, 8-core SPMD.

Sharding: tensor-parallel over the 16 query heads (2 heads per core).
MQA K/V (single head) is computed redundantly on every core. out_proj is
row-parallel: each core computes a partial [S, HID] output from its
256-wide slice of head dims; the all-reduce is the host-side gather sum.

Math notes (exact reformulations of the reference):
  * softmax without max-subtraction (scores ~ N(0,1), no overflow risk):
      a1 = exp(s1)/rowsum(exp(s1))
  * a = a1 - lam*a2 has rowsum exactly (1-lam), so the renorm divisor
    Z = (1-lam)+1e-8 is a constant -> folded into v_w on the host.
  * relu(a/Z) = relu(a)/Z since Z > 0.
  * 1/sqrt(head_dim) folded into q weights on the host (rope is a
    rotation, commutes with scaling).

Device layout: everything flows in "transposed" [feature, seq] form so
the tensor engine (which contracts over the partition dim) never needs
an activation transpose, except the post-softmax weights `a`, which are
cast to bf16 and transposed 128x128 via the DMA xbar.
"""

import math
from contextlib import ExitStack

import numpy as np

import concourse.bass as bass
import concourse.bacc as bacc
import concourse.tile as tile
from concourse import mybir
from concourse.bass_utils import run_bass_kernel_spmd

S = 2048          # sequence length
HID = 2048        # hidden dim
HEADS = 16
D = 128           # head dim
NCORES = 8
HPC = HEADS // NCORES   # heads per core = 2
LAM = 0.5
NQB = S // 128    # query blocks of 128
NHT = HID // 128  # hidden-dim tiles of 128

F32 = mybir.dt.float32
F32R = mybir.dt.float32r
BF16 = mybir.dt.bfloat16
AF = mybir.ActivationFunctionType
OP = mybir.AluOpType

NEG_MASK = -1.0e9
DEBUG_DUMPS = False


def _emit(ctx, tc, xT, wq1, wq2, wk, wv, cosT, sinT, prot, mneg, ow, y, klens, dbg=None):
    nc = tc.nc

    # ---------------- persistent tiles ----------------
    persist = ctx.enter_context(tc.tile_pool(name="persist", bufs=1))
    cos_s = persist.tile([128, S], F32, tag="cos")
    sin_s = persist.tile([128, S], F32, tag="sin")
    nc.sync.dma_start(out=cos_s, in_=cosT)
    nc.sync.dma_start(out=sin_s, in_=sinT)

    # roped projections, [d=128, S] each (transposed form)
    q1r = [persist.tile([128, S], BF16, name=f"q1r{h}", tag=f"q1r{h}") for h in range(HPC)]
    q2r = [persist.tile([128, S], BF16, name=f"q2r{h}", tag=f"q2r{h}") for h in range(HPC)]
    kr = persist.tile([128, S], BF16, tag="kr")
    vt_bf = persist.tile([128, S], BF16, tag="vt_bf")        # v^T, bf16
    v_nat = persist.tile([128, NQB, 128], BF16, tag="v_nat")  # v natural [k, d]

    # ---------------- stage A: projections + rope ----------------
    NCH = 4
    CW = S // NCH  # 512
    dram = ctx.enter_context(tc.tile_pool(name="dram", bufs=5, space="DRAM"))
    with tc.tile_pool(name="wpool", bufs=1) as wp, \
         tc.tile_pool(name="xpool", bufs=2) as xp, \
         tc.tile_pool(name="ropetmp", bufs=2) as rtp, \
         tc.tile_pool(name="projpsum", bufs=4, space="PSUM") as pp:
        wq1_s = wp.tile([128, NHT, HPC * D], BF16, tag="wq1")
        wq2_s = wp.tile([128, NHT, HPC * D], BF16, tag="wq2")
        wk_s = wp.tile([128, NHT, D], BF16, tag="wk")
        wv_s = wp.tile([128, NHT, D], BF16, tag="wv")
        prot_s = wp.tile([128, D], BF16, tag="prot")
        nc.sync.dma_start(out=prot_s, in_=prot)
        for dst, srcw in ((wq1_s, wq1), (wq2_s, wq2), (wk_s, wk), (wv_s, wv)):
            wsrc = srcw.rearrange("(t p) d -> p t d", p=128)
            for g in range(0, NHT, 8):
                nc.sync.dma_start(out=dst[:, g:g + 8, :], in_=wsrc[:, g:g + 8, :])

        for c in range(NCH):
            sl = slice(c * CW, (c + 1) * CW)
            xt = xp.tile([128, NHT, CW], BF16, tag="xt")
            xin = xT[:, sl].rearrange("(t p) s -> p t s", p=128)
            for g in range(0, NHT, 4):
                nc.sync.dma_start(out=xt[:, g:g + 4, :], in_=xin[:, g:g + 4, :])
            targets = []
            for h in range(HPC):
                targets.append((wq1_s, h * D, q1r[h], True))
                targets.append((wq2_s, h * D, q2r[h], True))
            targets.append((wk_s, 0, kr, True))
            targets.append((wv_s, 0, None, False))
            for (w_s, d0, dest, do_rope) in targets:
                ps = pp.tile([128, CW], F32, tag="ps")
                for t in range(NHT):
                    nc.tensor.matmul(
                        ps,
                        lhsT=w_s[:, t, d0:d0 + D],
                        rhs=xt[:, t, :],
                        start=(t == 0),
                        stop=(t == NHT - 1),
                    )
                if do_rope:
                    # rope in [d, s] layout: out = q*cos + rot_half(q)*sin.
                    # rot_half is a signed half-swap along the PARTITION dim;
                    # cross-partition reads are illegal on the vector engines,
                    # so apply it as a signed permutation matmul on PE.
                    q_sb = rtp.tile([128, CW], BF16, tag="qsb")
                    nc.scalar.copy(out=q_sb, in_=ps)
                    rot_ps = pp.tile([128, CW], F32, tag="rot")
                    nc.tensor.matmul(rot_ps, lhsT=prot_s, rhs=q_sb,
                                     start=True, stop=True)
                    m = rtp.tile([128, CW], F32, tag="m")
                    nc.vector.tensor_tensor(out=m, in0=q_sb, in1=cos_s[:, sl], op=OP.mult)
                    n = rtp.tile([128, CW], F32, tag="n")
                    nc.vector.tensor_tensor(out=n, in0=rot_ps, in1=sin_s[:, sl], op=OP.mult)
                    nc.vector.tensor_tensor(out=dest[:, sl], in0=m, in1=n, op=OP.add)
                else:
                    nc.scalar.copy(out=vt_bf[:, sl], in_=ps)  # cast f32 -> bf16

        # v^T [d, s] -> v natural [s, d]: bounce through DRAM (the xbar
        # transpose is only correct with a DRAM source on HW), one strip
        # store (gpsimd SWDGE, keeps the SP sequencer free) + per-tile
        # transpose loads.
        vt_dram = dram.tile([128, S], BF16, tag="vt_dram")
        nc.gpsimd.dma_start(out=vt_dram, in_=vt_bf)
        for t in range(NQB):
            nc.sync.dma_start(
                out=v_nat[:, t, :], in_=vt_dram[:, t * 128:(t + 1) * 128],
                transpose=True,
            )

    # ---------------- stage B: attention + out_proj ----------------
    with tc.tile_pool(name="bconst", bufs=1) as bc, \
         tc.tile_pool(name="strips", bufs=(3 if dbg is not None else 5)) as sp, \
         tc.tile_pool(name="smallp", bufs=6) as smp, \
         tc.tile_pool(name="atp", bufs=40) as atp, \
         tc.tile_pool(name="yp", bufs=2) as yp, \
         tc.tile_pool(name="spsum", bufs=3, space="PSUM") as spp, \
         tc.tile_pool(name="apsum", bufs=2, space="PSUM") as app:
        ow_s = bc.tile([128, HPC, HID], BF16, tag="ow")
        nc.sync.dma_start(out=ow_s, in_=ow.rearrange("(h p) e -> p h e", p=128))
        mneg_s = bc.tile([128, S], F32, tag="mneg")
        nc.sync.dma_start(out=mneg_s, in_=mneg)

        def emit_out_proj(qb, attnT):
            # row-parallel partial: y[qb] = sum_h attnT_h.T @ ow_h
            ysb = yp.tile([128, HID], F32, tag="ysb")
            for ec in range(2):
                yps = spp.tile([128, 1024], F32, tag="s")
                for sub in range(2):
                    e0 = ec * 1024 + sub * 512
                    for h in range(HPC):
                        nc.tensor.matmul(
                            yps[:, sub * 512:(sub + 1) * 512],
                            lhsT=attnT[h],
                            rhs=ow_s[:, h, e0:e0 + 512],
                            start=(h == 0),
                            stop=(h == HPC - 1),
                        )
                if ec == 0:
                    nc.scalar.copy(out=ysb[:, 0:1024], in_=yps)
                else:
                    nc.vector.tensor_copy(out=ysb[:, 1024:2048], in_=yps)
            nc.sync.dma_start(out=y[qb * 128:(qb + 1) * 128, :], in_=ysb)

        pending = None  # (qb, attnT) deferred one iteration for PE overlap
        for qb in range(NQB):
            klen = klens[qb]
            nkt = klen // 128
            nch = (klen + 1023) // 1024
            attnT = []
            aT_lists = []
            for h in range(HPC):
                p1 = sp.tile([128, S], BF16, tag="p1")
                p2 = sp.tile([128, S], BF16, tag="p2")
                r1c = smp.tile([128, 2], F32, tag="r1c")
                r2c = smp.tile([128, 2], F32, tag="r2c")
                q1T = q1r[h][:, qb * 128:(qb + 1) * 128]
                q2T = q2r[h][:, qb * 128:(qb + 1) * 128]
                for c in range(nch):
                    k0 = c * 1024
                    kc = min(1024, klen - k0)
                    for (qT, pstrip, rc) in ((q1T, p1, r1c), (q2T, p2, r2c)):
                        sps = spp.tile([128, 1024], F32, tag="s")
                        for sub in range(0, kc, 512):
                            sw = min(512, kc - sub)
                            nc.tensor.matmul(
                                sps[:, sub:sub + sw],
                                lhsT=qT,
                                rhs=kr[:, k0 + sub:k0 + sub + sw],
                                start=True,
                                stop=True,
                            )
                        if c == nch - 1:
                            # mask for the last (possibly partial) k block
                            dc = kc - 128
                            nc.vector.tensor_tensor(
                                out=sps[:, dc:dc + 128],
                                in0=sps[:, dc:dc + 128],
                                in1=mneg_s[:, qb * 128:(qb + 1) * 128],
                                op=OP.add,
                            )
                        if dbg is not None and qb == 15 and h == 0 and c == 0 and pstrip is p1:
                            sdump = sp.tile([128, 1024], F32, name="sdump", tag="sdump", bufs=1)
                            nc.vector.tensor_copy(out=sdump, in_=sps[:, :kc])
                            nc.sync.dma_start(out=dbg["dbg_s1"], in_=sdump)
                        nc.scalar.activation(
                            out=pstrip[:, k0:k0 + kc],
                            in_=sps[:, :kc],
                            func=AF.Exp,
                            accum_out=rc[:, c:c + 1],
                        )
                # rowsums -> reciprocals
                c1 = smp.tile([128, 1], F32, tag="c1")
                c2m = smp.tile([128, 1], F32, tag="c2m")
                if nch > 1:
                    nc.vector.tensor_tensor(
                        out=r1c[:, 0:1], in0=r1c[:, 0:1], in1=r1c[:, 1:2], op=OP.add
                    )
                    nc.vector.tensor_tensor(
                        out=r2c[:, 0:1], in0=r2c[:, 0:1], in1=r2c[:, 1:2], op=OP.add
                    )
                nc.vector.reciprocal(out=c1, in_=r1c[:, 0:1])
                nc.vector.reciprocal(out=c2m, in_=r2c[:, 0:1])
                nc.vector.tensor_scalar_mul(c2m, c2m, -LAM)
                # w = relu(p1*c1 - lam*p2*c2)  (bf16 out)
                nc.vector.tensor_scalar_mul(p1[:, :klen], p1[:, :klen], c1)
                nc.vector.scalar_tensor_tensor(
                    out=p2[:, :klen],
                    in0=p2[:, :klen],
                    scalar=c2m,
                    in1=p1[:, :klen],
                    op0=OP.mult,
                    op1=OP.add,
                )
                if dbg is not None and qb == 15 and h == 0:
                    pd = sp.tile([128, S], F32, name="pd", tag="pdump", bufs=1)
                    nc.scalar.copy(out=pd, in_=p1[:, :])
                    nc.sync.dma_start(out=dbg["dbg_p1"], in_=pd)
                    pd2 = sp.tile([128, S], F32, name="pd2", tag="pdump2", bufs=1)
                    nc.scalar.copy(out=pd2, in_=p2[:, :])
                    nc.sync.dma_start(out=dbg["dbg_p2"], in_=pd2)
                    rd = smp.tile([128, 4], F32, name="rd", tag="rdump", bufs=1)
                    nc.vector.tensor_copy(out=rd[:, 0:1], in_=r1c[:, 0:1])
                    nc.vector.tensor_copy(out=rd[:, 1:2], in_=r2c[:, 0:1])
                    nc.vector.tensor_copy(out=rd[:, 2:3], in_=c1)
                    nc.vector.tensor_copy(out=rd[:, 3:4], in_=c2m)
                    nc.sync.dma_start(out=dbg["dbg_r"], in_=rd)
                    qd = smp.tile([128, 128], F32, name="qd", tag="qdump", bufs=1)
                    nc.scalar.copy(out=qd, in_=q1T)
                    nc.sync.dma_start(out=dbg["dbg_q1T"], in_=qd)
                    krd = sp.tile([128, S], F32, name="krd", tag="krdump", bufs=1)
                    nc.scalar.copy(out=krd, in_=kr)
                    nc.sync.dma_start(out=dbg["dbg_kr"], in_=krd)
                w_bf = sp.tile([128, S], BF16, tag="wbf")
                nc.scalar.activation(out=w_bf[:, :klen], in_=p2[:, :klen], func=AF.Relu)
                # transpose w via DRAM bounce (xbar transpose needs DRAM src):
                # strip store on gpsimd SWDGE, per-tile transpose loads on SP
                w_dram = dram.tile([128, S], BF16, tag="w_dram")
                nc.gpsimd.dma_start(out=w_dram[:, :klen], in_=w_bf[:, :klen])
                # attn_out^T[d, q] = sum_k v[k, d] * w^T[k, q]
                aTs = []
                for kt in range(nkt):
                    aT = atp.tile([128, 128], BF16, tag="aT")
                    nc.sync.dma_start(
                        out=aT, in_=w_dram[:, kt * 128:(kt + 1) * 128], transpose=True
                    )
                    aTs.append(aT)
                aT_lists.append(aTs)
            # deferred a@v: both heads' transposes issue before either matmul
            # group runs, hiding the DRAM-bounce tail behind score work
            for h in range(HPC):
                aps = app.tile([128, 128], F32, tag="attn")
                for kt in range(nkt):
                    nc.tensor.matmul(
                        aps,
                        lhsT=v_nat[:, kt, :],
                        rhs=aT_lists[h][kt],
                        start=(kt == 0),
                        stop=(kt == nkt - 1),
                    )
                at_s = smp.tile([128, 128], BF16, name=f"attnT{h}", tag=f"attnT{h}")
                nc.scalar.copy(out=at_s, in_=aps)
                if dbg is not None and qb == 15 and h == 0:
                    wd = sp.tile([128, S], F32, name="wd", tag="wdump", bufs=1)
                    nc.scalar.copy(out=wd, in_=w_bf[:, :])
                    nc.sync.dma_start(out=dbg["dbg_w"], in_=wd)
                    atd = smp.tile([128, 128], F32, name="atd", tag="atdump", bufs=1)
                    nc.scalar.copy(out=atd, in_=aT)
                    nc.sync.dma_start(out=dbg["dbg_aT"], in_=atd)
                    attd = smp.tile([128, 128], F32, name="attd", tag="attdump", bufs=1)
                    nc.scalar.copy(out=attd, in_=aps)
                    nc.sync.dma_start(out=dbg["dbg_attnT"], in_=attd)
                attnT.append(at_s)
            if pending is not None:
                emit_out_proj(*pending)
            pending = (qb, attnT)
        emit_out_proj(*pending)


def _build(klens):
    nc = bacc.Bacc("TRN2", target_bir_lowering=False, debug=False)
    xT = nc.dram_tensor("xT", [HID, S], BF16, kind="ExternalInput").ap()
    wq1 = nc.dram_tensor("wq1", [HID, HPC * D], BF16, kind="ExternalInput").ap()
    wq2 = nc.dram_tensor("wq2", [HID, HPC * D], BF16, kind="ExternalInput").ap()
    wk = nc.dram_tensor("wk", [HID, D], BF16, kind="ExternalInput").ap()
    wv = nc.dram_tensor("wv", [HID, D], BF16, kind="ExternalInput").ap()
    cosT = nc.dram_tensor("cosT", [D, S], F32, kind="ExternalInput").ap()
    prot = nc.dram_tensor("prot", [D, D], BF16, kind="ExternalInput").ap()
    sinT = nc.dram_tensor("sinT", [D, S], F32, kind="ExternalInput").ap()
    mneg = nc.dram_tensor("mneg", [128, S], F32, kind="ExternalInput").ap()
    ow = nc.dram_tensor("ow", [HPC * D, HID], BF16, kind="ExternalInput").ap()
    y = nc.dram_tensor("y", [S, HID], F32, kind="ExternalOutput").ap()
    dbg = None
    if DEBUG_DUMPS:
        dbg = {
            "dbg_p1": nc.dram_tensor("dbg_p1", [128, S], F32, kind="ExternalOutput").ap(),
            "dbg_p2": nc.dram_tensor("dbg_p2", [128, S], F32, kind="ExternalOutput").ap(),
            "dbg_r": nc.dram_tensor("dbg_r", [128, 4], F32, kind="ExternalOutput").ap(),
            "dbg_w": nc.dram_tensor("dbg_w", [128, S], F32, kind="ExternalOutput").ap(),
            "dbg_aT": nc.dram_tensor("dbg_aT", [128, 128], F32, kind="ExternalOutput").ap(),
            "dbg_attnT": nc.dram_tensor("dbg_attnT", [128, 128], F32, kind="ExternalOutput").ap(),
            "dbg_q1T": nc.dram_tensor("dbg_q1T", [128, 128], F32, kind="ExternalOutput").ap(),
            "dbg_kr": nc.dram_tensor("dbg_kr", [128, S], F32, kind="ExternalOutput").ap(),
            "dbg_s1": nc.dram_tensor("dbg_s1", [128, 1024], F32, kind="ExternalOutput").ap(),
        }
    with ExitStack() as ctx:
        tc = ctx.enter_context(tile.TileContext(nc))
        _emit(ctx, tc, xT, wq1, wq2, wk, wv, cosT, sinT, prot, mneg, ow, y, klens, dbg)
    nc.compile()
    return nc


_RUNNER_CACHE = {}
LAST_RUN = None
LAST_EXEC = None  # (runner, dev_args) for timing reuse


class _Runner:
    """Mirrors bass2jax.run_bass_via_pjrt's multi-core path, but caches the
    jitted executable and keeps inputs reusable (no donation) so repeated
    timed executions don't recompile or re-upload."""

    def __init__(self, nc, n_cores):
        import jax
        from jax.sharding import Mesh, PartitionSpec
        from jax.experimental.shard_map import shard_map
        from concourse import bass2jax, mybir as mb

        bass2jax.install_neuronx_cc_hook()
        self.nc = nc
        self.n_cores = n_cores
        partition_name = (
            nc.partition_id_tensor.name if nc.partition_id_tensor else None
        )
        in_names, out_names, out_avals, zero_outs = [], [], [], []
        for alloc in nc.m.functions[0].allocations:
            if not isinstance(alloc, mb.MemoryLocationSet):
                continue
            name = alloc.memorylocations[0].name
            if alloc.kind == "ExternalInput":
                if name != partition_name:
                    in_names.append(name)
            elif alloc.kind == "ExternalOutput":
                out_names.append(name)
                shape = tuple(alloc.tensor_shape)
                dtype = mb.dt.np(alloc.dtype)
                out_avals.append(jax.core.ShapedArray(shape, dtype))
                zero_outs.append(np.zeros(shape, dtype))
        self.in_names = list(in_names)
        self.out_names = out_names
        self.out_avals = out_avals
        self.zero_outs = zero_outs
        n_params = len(in_names)
        all_names = list(in_names + out_names)
        if partition_name is not None:
            all_names.append(partition_name)
        all_names = tuple(all_names)

        def _body(*args):
            operands = list(args)
            if partition_name is not None:
                operands.append(bass2jax.partition_id_tensor())
            outs = bass2jax._bass_exec_p.bind(
                *operands,
                out_avals=tuple(out_avals),
                in_names=all_names,
                out_names=tuple(out_names),
                lowering_input_output_aliases=(),
                sim_require_finite=True,
                sim_require_nnan=True,
                nc=nc,
            )
            return tuple(outs)

        self._body = _body
        devices = jax.devices()[:n_cores]
        self.mesh = Mesh(np.asarray(devices), ("core",))
        self.pspec = PartitionSpec("core")
        in_specs = (self.pspec,) * (n_params + len(out_names))
        out_specs = (self.pspec,) * len(out_names)
        self.fn = jax.jit(
            shard_map(_body, mesh=self.mesh, in_specs=in_specs,
                      out_specs=out_specs, check_rep=False),
            keep_unused=True,
        )

    def loop_fn(self, n):
        """Jitted function executing the kernel n times back-to-back on
        device (effect-ordered). Used to amortize the ~78 ms axon dispatch
        overhead when measuring true HW exec time."""
        import jax
        from jax.experimental.shard_map import shard_map

        if not hasattr(self, "_loop_fns"):
            self._loop_fns = {}
        if n not in self._loop_fns:
            body = self._body

            def _loop(*args):
                outs = None
                for _ in range(n):
                    outs = body(*args)
                return outs

            n_params = len(self.in_names)
            in_specs = (self.pspec,) * (n_params + len(self.out_names))
            out_specs = (self.pspec,) * len(self.out_names)
            self._loop_fns[n] = jax.jit(
                shard_map(_loop, mesh=self.mesh, in_specs=in_specs,
                          out_specs=out_specs, check_rep=False),
                keep_unused=True,
            )
        return self._loop_fns[n]

    def device_args(self, in_maps):
        import jax
        from jax.sharding import NamedSharding

        sharding = NamedSharding(self.mesh, self.pspec)
        concat = [
            np.concatenate([np.asarray(m[name]) for m in in_maps], axis=0)
            for name in self.in_names
        ]
        concat += [
            np.zeros((self.n_cores * z.shape[0], *z.shape[1:]), z.dtype)
            for z in self.zero_outs
        ]
        return [jax.device_put(a, sharding) for a in concat]

    def run(self, dev_args):
        import jax

        outs = self.fn(*dev_args)
        jax.block_until_ready(outs)
        return [
            {
                name: np.asarray(outs[i]).reshape(
                    self.n_cores, *self.out_avals[i].shape)[c]
                for i, name in enumerate(self.out_names)
            }
            for c in range(self.n_cores)
        ]


def _get_runner(klens):
    key = tuple(klens)
    if key not in _RUNNER_CACHE:
        _RUNNER_CACHE[key] = _Runner(_build(klens), NCORES)
    return _RUNNER_CACHE[key]


def measure_hw(n_long=96, n_short=8, reps=4):
    """True per-execution HW time via loop amortization: run the kernel
    n_long and n_short times in single dispatches; the slope removes the
    ~78 ms axon dispatch overhead."""
    import time
    import jax

    runner, dev_args = LAST_EXEC
    f_long = runner.loop_fn(n_long)
    f_short = runner.loop_fn(n_short)

    def timed(f):
        best = float("inf")
        for _ in range(reps):
            t0 = time.perf_counter()
            outs = f(*dev_args)
            jax.block_until_ready(outs)
            best = min(best, time.perf_counter() - t0)
        return best

    timed(f_short)  # warm both paths
    timed(f_long)
    t_short = timed(f_short)
    t_long = timed(f_long)
    per_exec = (t_long - t_short) / (n_long - n_short)
    if per_exec <= 0:
        per_exec = t_long / n_long  # upper bound when RPC noise dominates
    return per_exec, t_short, t_long


def _prep_mask(mask):
    """Per query-block: attended k extent (klen) and the additive mask for
    the last 128-wide k block. Requires every non-final block in range to
    be all-True (holds for causal and for all-ones masks)."""
    mask = np.asarray(mask).astype(bool)
    klens = []
    mneg = np.zeros((128, S), np.float32)
    for qb in range(NQB):
        rows = mask[qb * 128:(qb + 1) * 128, :]
        any_col = rows.any(axis=0)
        assert any_col.any(), f"query block {qb} attends nothing"
        last = int(np.nonzero(any_col)[0][-1])
        nkt = last // 128 + 1
        klen = nkt * 128
        klens.append(klen)
        blk = rows[:, (nkt - 1) * 128:klen]
        mneg[:, qb * 128:(qb + 1) * 128] = np.where(blk, 0.0, NEG_MASK)
        inner = rows[:, :(nkt - 1) * 128]
        if not inner.all():
            raise NotImplementedError(
                "mask has partial blocks before the final attended block; "
                "only causal / all-ones style masks are supported"
            )
    return klens, mneg


def host_prep(x, freqs_cos, freqs_sin, mask, q1_w, q2_w, k_w, v_w, out_w):
    """Host-side input marshalling: transpose/fold/shard. Returns
    (klens, in_maps)."""
    x = np.asarray(x, np.float32)
    assert x.shape == (1, S, HID)
    xT = np.ascontiguousarray(x[0].T)
    scale = 1.0 / math.sqrt(D)
    Z = (1.0 - LAM) + 1e-8

    cosT = np.ascontiguousarray(np.asarray(freqs_cos, np.float32).T)
    sinT = np.ascontiguousarray(np.asarray(freqs_sin, np.float32).T)
    # signed rotate-half as a matmul: rot = protM @ q with
    # protM[d, d+64] = -1 (d<64), protM[d, d-64] = +1 (d>=64); lhsT = protM.T
    protM = np.zeros((D, D), np.float32)
    for d in range(64):
        protM[d, d + 64] = -1.0
        protM[d + 64, d] = 1.0
    protT = np.ascontiguousarray(protM.T)

    klens, mneg = _prep_mask(mask)

    q1_w = np.asarray(q1_w, np.float32) * scale
    q2_w = np.asarray(q2_w, np.float32) * scale
    k_w = np.ascontiguousarray(np.asarray(k_w, np.float32))
    v_w = np.ascontiguousarray(np.asarray(v_w, np.float32) / Z)
    out_w = np.asarray(out_w, np.float32)

    import ml_dtypes
    bf = ml_dtypes.bfloat16
    xT = xT.astype(bf)
    cost_ = cosT; sint_ = sinT
    k_w = k_w.astype(bf)
    v_w = v_w.astype(bf)
    protT = protT.astype(bf)
    in_maps = []
    for c in range(NCORES):
        h0 = c * HPC * D
        in_maps.append({
            "xT": xT,
            "wq1": np.ascontiguousarray(q1_w[:, h0:h0 + HPC * D]).astype(bf),
            "wq2": np.ascontiguousarray(q2_w[:, h0:h0 + HPC * D]).astype(bf),
            "wk": k_w,
            "wv": v_w,
            "cosT": cosT,
            "sinT": sinT,
            "prot": protT,
            "mneg": mneg,
            "ow": np.ascontiguousarray(out_w[h0:h0 + HPC * D, :]).astype(bf),
        })
    return klens, in_maps


def kernel(x, freqs_cos, freqs_sin, mask, q1_w, q2_w, k_w, v_w, out_w):
    global LAST_RUN, LAST_EXEC
    klens, in_maps = host_prep(
        x, freqs_cos, freqs_sin, mask, q1_w, q2_w, k_w, v_w, out_w)
    runner = _get_runner(klens)
    dev_args = runner.device_args(in_maps)
    LAST_EXEC = (runner, dev_args)
    results = runner.run(dev_args)
    LAST_RUN = results
    y = results[0]["y"].astype(np.float32)
    for c in range(1, NCORES):
        y = y + results[c]["y"]
    return y.reshape(1, S, HID)

